# revision 27
# baseline (speedup 1.0000x reference)
"""Trainium2 Bass kernel for CLIP-style symmetric contrastive loss.

Problem: image_features [8192, 1024] f32, text_features [8192, 1024] f32.
  loss = 0.5 * (CE(logits, diag) + CE(logits.T, diag)),
  logits = cosine_similarity(img, txt) / 0.07.

Distribution: shard image rows across 8 NeuronCores. Each core m computes the
slab S_m = txt_n @ img_n[m].T  ([8192 j, 1024 i]) — text rows on PSUM
partitions, the core's own image rows on the free axis — using fp8(e4m3)
DoubleRow matmuls (K=256 per pass, 0.5 cycles/row). exp(S - C) is reduced
along the free axis (ACT accum -> per-text-row colsum partials) and
elementwise-accumulated across j-blocks (DVE bf16 -> rowsum for the core's
own image rows). The colsum AllReduce is split in two so most of it overlaps
the tail of the main loop.

Normalization: each core computes 1/||.|| for its OWN 1024 text rows (row-
major ACT square+accum, issued before anything else so the [8192] AllReduce
of reciprocal norms overlaps the big operand loads), a gpsimd
partition_broadcast replicates them across partitions, and a DVE+GpSimd pass
rescales the host-shipped raw fp8 text in place. sqrt(1/T) is folded into
both image and text scales so ACT-Exp runs with scale=1.

Math (C = 1/T upper-bounds every logit, so exp(S - C) <= 1 is stable):
  loss = C + (R + L - (2/T) * Draw) / (2N)
    R    = sum_i log sum_j exp(S_ij - C)   (own-i partials, AllReduced)
    L    = sum_j log sum_i exp(S_ij - C)   (colsums AllReduced, ln locally)
    Draw = sum_i cos(img_i, txt_i)         (f32, own rows, AllReduced)
"""
import threading
from contextlib import ExitStack

import ml_dtypes
import numpy as np

import concourse.bacc as bacc
import concourse.bass as bass
import concourse.bass_isa as bass_isa
import concourse.mybir as mybir
import concourse.tile as tile
from concourse.bass_utils import run_bass_kernel_spmd

F32 = mybir.dt.float32
BF16 = mybir.dt.bfloat16
FP8 = mybir.dt.float8e4
AF = mybir.ActivationFunctionType
ALU = mybir.AluOpType
DR = mybir.MatmulPerfMode.DoubleRow

N_CORES = 8
N = 8192
D = 1024
TEMPERATURE = 0.07


def build_nc(n=N, d=D, n_cores=N_CORES, no_collective=False):
    inv_t = float(1.0 / TEMPERATURE)
    rows = n // n_cores                      # image/text rows per core (1024)
    P = 128
    rp = rows // P                           # row tiles per core (8)
    kt = d // P                              # 128-deep k tiles (8)
    nc = bacc.Bacc("TRN2", target_bir_lowering=False, debug=False,
                   num_devices=n_cores)
    img = nc.dram_tensor("img", [rows, d], BF16, kind="ExternalInput").ap()
    txt_own = nc.dram_tensor("txt_own", [rows, d], BF16, kind="ExternalInput").ap()
    imgT = nc.dram_tensor("imgT", [kt * P, rows], BF16, kind="ExternalInput").ap()
    txt8 = nc.dram_tensor("txt8", [kt * P, n], FP8, kind="ExternalInput").ap()
    ones_b = nc.dram_tensor("ones_b", [P, P], BF16, kind="ExternalInput").ap()
    rmask = nc.dram_tensor("rmask", [P, n // P], F32, kind="ExternalInput").ap()
    out = nc.dram_tensor("out", [1, 1], F32, kind="ExternalOutput").ap()

    with tile.TileContext(nc) as tc:
        _body(tc, img, txt_own, imgT, txt8, ones_b, rmask, out,
              n=n, d=d, rows=rows, P=P, rp=rp, kt=kt, inv_t=inv_t,
              n_cores=n_cores, no_collective=no_collective)
    nc.compile()
    return nc


def _body(tc, img, txt_own, imgT, txt8d, ones_b, rmask, out, *, n, d, rows, P,
          rp, kt, inv_t, n_cores, no_collective):
    nc = tc.nc
    cexp = inv_t                 # stabilizer: max possible logit
    sqs = float(np.sqrt(inv_t))  # sqrt(1/T), folded into both scales
    jB = n // P                  # 64 j-blocks of 128 text rows
    JC = 8                       # text scale chunks
    jc_w = n // JC               # 1024 columns per scale chunk
    CI = rows // 512             # image free chunks (2)
    SPLIT = 40                   # j-blocks covered by the first AllReduce
    nA = SPLIT * P               # 6144

    with ExitStack() as ctx:
        persist = ctx.enter_context(tc.tile_pool(name="persist", bufs=1))
        stage_f = ctx.enter_context(tc.tile_pool(name="stage_f", bufs=3))
        stage_b = ctx.enter_context(tc.tile_pool(name="stage_b", bufs=4))
        v1 = ctx.enter_context(tc.tile_pool(name="v1", bufs=4))
        r1p = ctx.enter_context(tc.tile_pool(name="r1p", bufs=4))
        keepP = ctx.enter_context(tc.tile_pool(name="keepP", bufs=8))
        exp_p = ctx.enter_context(tc.tile_pool(name="exp_p", bufs=8))
        ctp = ctx.enter_context(tc.tile_pool(name="ctp", bufs=4))
        psum = ctx.enter_context(tc.tile_pool(name="psum", bufs=8, space="PSUM"))
        dram = ctx.enter_context(tc.tile_pool(name="dram", bufs=1, space="DRAM"))

        txt8s = persist.tile([P, kt, n], FP8, tag="txt8")      # 64KB/part
        img8 = persist.tile([P, kt, rows], FP8, tag="img8")    # 8KB
        imgTs = persist.tile([P, kt, rows], BF16, tag="imgTs")  # 16KB
        rcpt = persist.tile([P, n], F32, tag="rcpt")           # 32KB
        rcpi = persist.tile([P, rows], F32, tag="rcpi")        # 4KB
        acc = persist.tile([P, CI, 512], BF16, tag="acc")      # 2KB
        colacc = persist.tile([P, jB], F32, tag="colacc")
        vecs = persist.tile([P, 40], F32, tag="vecs")
        ones_sb = persist.tile([P, P], BF16, tag="ones")
        ebias = persist.tile([P, 1], F32, tag="ebias")
        csA = persist.tile([P, SPLIT], F32, tag="csA")
        csB = persist.tile([P, jB - SPLIT], F32, tag="csB")
        lnA = persist.tile([P, SPLIT], F32, tag="lnA")
        lnB = persist.tile([P, jB - SPLIT], F32, tag="lnB")

        ibuf = dram.tile([1, rows], F32, tag="ibuf")
        cbufA = dram.tile([1, nA], F32, tag="cbufA")
        cbufA_out = dram.tile([1, nA], F32, tag="cbufA_out", addr_space="Shared")
        cbufB = dram.tile([1, n - nA + 2], F32, tag="cbufB")
        cbufB_out = dram.tile([1, n - nA + 2], F32, tag="cbufB_out",
                              addr_space="Shared")

        # vecs columns: 0..7 img sumsq | 8..15 txt sumsq | 16..23 diag dot
        # 24..31 irc/trc | 32.. scalars
        ISQ, TSQ, DOT, SC = 0, 8, 16, 32


        nc.sync.dma_start(ones_sb[:], ones_b[:])
        nc.gpsimd.memset(ebias[:], float(-cexp))
        for k in range(kt):
            nc.sync.dma_start(txt8s[:, k, :], txt8d[k * P:(k + 1) * P, :])
        for k in range(kt):
            nc.sync.dma_start(imgTs[:, k, :], imgT[k * P:(k + 1) * P, :])

        # --- Own-text norms (exact-ish, for the diagonal term only) --------
        keeps = []
        for t in range(rp):
            keep_t = keepP.tile([P, d], BF16, tag="keep")
            nc.sync.dma_start(keep_t[:], txt_own[t * P:(t + 1) * P, :])
            sq = stage_b.tile([P, d], BF16, tag="sq")
            nc.scalar.activation(sq[:], keep_t[:], AF.Square,
                                 accum_out=vecs[:, TSQ + t:TSQ + t + 1])
            keeps.append(keep_t)

        irc = vecs[:, 24:24 + rp]
        trc = vecs[:, 24 + rp:24 + 2 * rp]
        nrm = v1.tile([P, 2 * rp], F32, tag="nrm")
        nc.scalar.activation(nrm[:, 0:rp], vecs[:, TSQ:TSQ + rp], AF.Sqrt)
        nc.vector.reciprocal(trc, nrm[:, 0:rp])
        # --- Image prep -----------------------------------------------------
        for t in range(rp):
            img_raw = stage_f.tile([P, d], BF16, tag="stage")
            nc.sync.dma_start(img_raw[:], img[t * P:(t + 1) * P, :])
            sq2 = stage_b.tile([P, d], BF16, tag="sq")
            nc.scalar.activation(sq2[:], img_raw[:], AF.Square,
                                 accum_out=vecs[:, ISQ + t:ISQ + t + 1])
            dots = stage_b.tile([P, d], BF16, tag="sq")
            nc.vector.tensor_tensor(dots[:], img_raw[:], keeps[t][:],
                                    ALU.mult)
            nc.vector.tensor_reduce(vecs[:, DOT + t:DOT + t + 1], dots[:],
                                    axis=mybir.AxisListType.X, op=ALU.add)

        nc.scalar.activation(nrm[:, rp:2 * rp], vecs[:, ISQ:ISQ + rp], AF.Sqrt)
        nc.vector.reciprocal(irc, nrm[:, rp:2 * rp])
        ircs = v1.tile([P, rp], F32, tag="v1s")
        nc.vector.tensor_scalar_mul(ircs[:], irc, sqs)

        # diag cosine partial
        dg = v1.tile([P, rp + 1], F32, tag="v1s")
        nc.vector.tensor_tensor(dg[:, 0:rp], vecs[:, DOT:DOT + rp], irc,
                                ALU.mult)
        nc.vector.tensor_tensor(dg[:, 0:rp], dg[:, 0:rp], trc, ALU.mult)
        nc.vector.tensor_reduce(dg[:, rp:rp + 1], dg[:, 0:rp],
                                axis=mybir.AxisListType.X, op=ALU.add)
        nc.gpsimd.partition_all_reduce(vecs[:, SC + 1:SC + 2], dg[:, rp:rp + 1],
                                       channels=P, reduce_op=bass_isa.ReduceOp.add)

        # image reciprocal norms -> all partitions (DRAM bounce + broadcast)
        nc.sync.dma_start(
            ibuf[0:1, :].rearrange("a (x p) -> (a p) x", p=P), ircs[:])
        i1 = r1p.tile([1, rows], F32, tag="r1")
        nc.sync.dma_start(i1[:], ibuf[:])
        nc.gpsimd.partition_broadcast(rcpi[:], i1[:])

        # Big operand loads on the Activation queue, explicitly gated behind
        # the collective's data phase so they cannot starve its DMA access.
        for k in range(kt):
            nc.vector.tensor_tensor(img8[:, k, :], imgTs[:, k, :], rcpi[:],
                                    ALU.mult)

        # --- Main loop. Text norms are computed locally per 512-col chunk:
        # ACT squares -> PE ones-matmul partition reduce -> sqrt(x/T) on one
        # partition -> reciprocal -> gpsimd broadcast -> DVE scale to unit
        # fp8. Interleaved per jc so the ACT queue stays in flow order.
        for jc in range(JC):
            sl = slice(jc * jc_w, (jc + 1) * jc_w)
            for c2 in range(2):
                csl = slice(jc * jc_w + c2 * 512, jc * jc_w + (c2 + 1) * 512)
                ssq = psum.tile([P, 512], F32, tag="mm")
                for k in range(kt):
                    sqc = stage_b.tile([P, d], BF16, tag="sq")
                    nc.scalar.activation(sqc[:, 0:512], txt8s[:, k, csl],
                                         AF.Square)
                    nc.tensor.matmul(ssq[:], ones_sb[:], sqc[:, 0:512],
                                     start=(k == 0), stop=(k == kt - 1))
                snrm = r1p.tile([1, jc_w], F32, tag="r1")
                nc.scalar.activation(snrm[0:1, 0:512], ssq[0:1, :], AF.Sqrt,
                                     scale=float(1.0 / inv_t))
                rnc = r1p.tile([1, jc_w], F32, tag="r1")
                nc.vector.reciprocal(rnc[0:1, 0:512], snrm[0:1, 0:512])
                nc.gpsimd.partition_broadcast(rcpt[:, csl], rnc[0:1, 0:512])
            for k in range(kt):
                nc.vector.tensor_tensor(txt8s[:, k, sl], txt8s[:, k, sl],
                                        rcpt[:, sl], ALU.mult)
            for jb in range(jc * (jB // JC), (jc + 1) * (jB // JC)):
                mms = []
                for _ci in range(CI):
                    mm_t = psum.tile([P, 512], F32, tag="mm")
                    mms.append(mm_t)
                for t in range(kt // 2):
                    for ci in range(CI):
                        nc.tensor.matmul(
                            mms[ci][:],
                            txt8s[:, 2 * t:2 * t + 2, jb * P:(jb + 1) * P],
                            img8[:, 2 * t:2 * t + 2, ci * 512:(ci + 1) * 512],
                            start=(t == 0), stop=(t == kt // 2 - 1),
                            perf_mode=DR)
                ctmp = ctp.tile([P, CI], F32, tag="ct")
                for ci in range(CI):
                    ex = exp_p.tile([P, 512], BF16, tag="exp")
                    nc.scalar.activation(ex[:], mms[ci][:], AF.Exp,
                                         bias=ebias[:, 0:1], scale=1.0,
                                         accum_out=ctmp[:, ci:ci + 1])
                    if jb == 0:
                        nc.vector.tensor_copy(acc[:, ci, :], ex[:])
                    else:
                        nc.vector.tensor_tensor(acc[:, ci, :], acc[:, ci, :],
                                                ex[:], ALU.add)
                nc.vector.tensor_tensor(colacc[:, jb:jb + 1], ctmp[:, 0:1],
                                        ctmp[:, 1:2], ALU.add)
            if jb == SPLIT - 1:
                # first colsum AllReduce overlaps the last two chunks
                nc.sync.dma_start(
                    cbufA[0:1, :].rearrange("a (x p) -> (a p) x", p=P),
                    colacc[:, 0:SPLIT])
                if no_collective:
                    nc.sync.dma_start(cbufA_out[:], cbufA[:])
                else:
                    nc.gpsimd.collective_compute(
                        "AllReduce", ALU.add,
                        replica_groups=[list(range(n_cores))],
                        ins=[cbufA[:].opt()], outs=[cbufA_out[:].opt()])
                nc.sync.dma_start(
                    csA[:],
                    cbufA_out[0:1, :].rearrange("a (p x) -> (a p) x", p=P))
                laA = ctp.tile([P, 1], F32, tag="laA")
                nc.scalar.activation(lnA[:], csA[:], AF.Ln, accum_out=laA[:])

        # --- Tail: rowsums, scalars, second AllReduce, finish ---------------
        rs = v1.tile([1, CI + 2], F32, tag="rs")
        for ci in range(CI):
            mm = psum.tile([P, 512], F32, tag="mm")
            nc.tensor.matmul(mm[:], ones_sb[:], acc[:, ci, :],
                             start=True, stop=True)
            lnr = exp_p.tile([1, 512], F32, tag="lnr")
            nc.scalar.activation(lnr[:], mm[0:1, :], AF.Ln,
                                 accum_out=rs[0:1, ci:ci + 1])
        sc2 = v1.tile([1, 2], F32, tag="sc2")
        nc.vector.tensor_tensor(sc2[0:1, 0:1], rs[0:1, 0:1], rs[0:1, 1:2],
                                ALU.add)                     # R partial
        nc.vector.tensor_copy(sc2[0:1, 1:2], vecs[0:1, SC + 1:SC + 2])

        nB = n - nA
        nc.sync.dma_start(
            cbufB[0:1, 0:nB].rearrange("a (x p) -> (a p) x", p=P),
            colacc[:, SPLIT:jB])
        nc.sync.dma_start(cbufB[0:1, nB:nB + 2], sc2[0:1, :])
        if no_collective:
            nc.sync.dma_start(cbufB_out[:], cbufB[:])
        else:
            nc.gpsimd.collective_compute(
                "AllReduce", ALU.add,
                replica_groups=[list(range(n_cores))],
                ins=[cbufB[:].opt()], outs=[cbufB_out[:].opt()])

        nc.sync.dma_start(
            csB[:], cbufB_out[0:1, 0:nB].rearrange("a (p x) -> (a p) x", p=P))
        laB = ctp.tile([P, 1], F32, tag="laB")
        nc.scalar.activation(lnB[:], csB[:], AF.Ln, accum_out=laB[:])
        laT = ctp.tile([P, 1], F32, tag="laT")
        nc.vector.tensor_tensor(laT[:], laA[:], laB[:], ALU.add)
        nc.gpsimd.partition_all_reduce(vecs[:, SC:SC + 1], laT[:],
                                       channels=P, reduce_op=bass_isa.ReduceOp.add)
        rd = v1.tile([1, 8], F32, tag="rd")
        nc.sync.dma_start(rd[0:1, 0:2], cbufB_out[0:1, nB:nB + 2])

        # loss = cexp + (R + L - (2/T) * Draw) / (2N)
        fin = v1.tile([1, 8], F32, tag="fin")
        nc.vector.tensor_tensor(fin[0:1, 0:1], rd[0:1, 0:1],
                                vecs[0:1, SC:SC + 1], ALU.add)
        nc.vector.tensor_scalar_mul(fin[0:1, 1:2], rd[0:1, 1:2],
                                    float(-2.0 * inv_t))
        nc.vector.tensor_tensor(fin[0:1, 2:3], fin[0:1, 0:1],
                                fin[0:1, 1:2], ALU.add)
        nc.scalar.activation(fin[0:1, 3:4], fin[0:1, 2:3], AF.Copy,
                             bias=float(cexp), scale=float(1.0 / (2 * n)))
        nc.sync.dma_start(out[0:1, 0:1], fin[0:1, 3:4])


def make_in_maps(image_features, text_features, n=N, d=D, n_cores=N_CORES):
    image_features = np.asarray(image_features, dtype=np.float32)
    text_features = np.asarray(text_features, dtype=np.float32)
    rows = n // n_cores
    P = 128
    kt = d // P
    txt8 = np.ascontiguousarray(text_features.T).astype(
        ml_dtypes.float8_e4m3).reshape(kt * P, n)
    ones_b = np.ones((P, P), dtype=ml_dtypes.bfloat16)
    maps = []
    for m in range(n_cores):
        img_sh = image_features[m * rows:(m + 1) * rows]
        imgT = np.ascontiguousarray(img_sh.T).astype(
            ml_dtypes.bfloat16).reshape(kt * P, rows)
        rmask = np.zeros((P, n // P), dtype=np.float32)
        rmask[:, m * (rows // P):(m + 1) * (rows // P)] = 1.0
        maps.append({
            "img": img_sh.astype(ml_dtypes.bfloat16),
            "txt_own": text_features[m * rows:(m + 1) * rows].astype(
                ml_dtypes.bfloat16),
            "imgT": imgT,
            "txt8": txt8,
            "ones_b": ones_b,
            "rmask": rmask,
        })
    return maps


_CACHE = {}
_LOCK = threading.Lock()


def _get_nc():
    with _LOCK:
        if "nc" not in _CACHE:
            _CACHE["nc"] = build_nc()
        return _CACHE["nc"]


def kernel(image_features, text_features):
    image_features = np.asarray(image_features, dtype=np.float32)
    text_features = np.asarray(text_features, dtype=np.float32)
    assert image_features.shape == (N, D) and text_features.shape == (N, D)
    nc = _get_nc()
    in_maps = make_in_maps(image_features, text_features)
    res = run_bass_kernel_spmd(nc, in_maps, list(range(N_CORES)))
    val = np.float32(res.results[0]["out"][0, 0])
    return np.array(val, dtype=np.float32)
